# revision 26
# baseline (speedup 1.0000x reference)
"""Trainium2 Bass kernel for a pre-LN causal decoder layer (MHA + SwiGLU).

Sharding: 2-way data parallel over batch x 4-way tensor parallel over heads.
Core c (of 8): batch b=c//4, group rank r=c%4, heads [4r, 4r+4).

fp8(e4m3) DoubleRow matmuls for QKV projections, QK^T, AV, and Wo (2-4x PE
throughput vs bf16); SwiGLU FFN stays bf16 for accuracy. LayerNorm is folded
into the projections: matmuls run on raw 16*x (fp8) and the per-token
(mean, rstd) fixup is applied at the PSUM->SBUF quantize stage; gamma and all
fp8 scale factors are folded into the weights on the host.

Attention emits one ReduceScatter chunk per 512-token J-block (4 chunks), so
the FFN's input (LN2 of attn+residual) is mostly ready when attention ends.
"""

import sys

sys.path.insert(0, "/opt/trn_rl_repo")

import numpy as np
import ml_dtypes

import concourse.bass as bass
import concourse.mybir as mybir
import concourse.tile as tile
from concourse import bacc
from concourse.bass_utils import run_bass_kernel_spmd
from concourse.masks import make_identity

BF16 = ml_dtypes.bfloat16
E4 = ml_dtypes.float8_e4m3
F32 = mybir.dt.float32
BF = mybir.dt.bfloat16
FP8 = mybir.dt.float8e4
DRM = mybir.MatmulPerfMode.DoubleRow
AF = mybir.ActivationFunctionType

B, T, C = 2, 2048, 1024
H, HS = 16, 64
HID = 2730
HIDP = 2816  # padded to 22*128
NF = HIDP // 128  # 22
HPC = 4  # heads per core
TLOC = T // 4  # 512 tokens owned post-RS
EPS = 1e-3
RG = [[0, 1, 2, 3], [4, 5, 6, 7]]
NT = T // 128  # 16 s-tiles
NJ = T // 512  # 4 J-blocks
NKC = C // 128  # 8 contraction chunks
NK2 = NKC // 2  # 4 DR contraction pairs

# fp8 scale factors
SX = 16.0    # x
SW = 64.0    # qkv weights
SMU = 512.0  # negmu
SGWS = SX * SW / SMU  # = 2; gws correction row
SSQ = 4.0    # x^2 (4*x^2 max ~92 < e4m3 max 224)
SQ = 32.0    # q (with 1/8 folded)
SK = 4.0     # k
SV = 4.0     # v
SP = 8.0     # exp probs
SWO = 64.0   # Wo
NEGF = -60.0 * SQ * SK  # mask fill in scaled-score units

_cache = {}


def _build(have_bw, sim=False, debug=False):
    nc = bacc.Bacc(None, target_bir_lowering=False, debug=False)
    xT = nc.declare_dram_parameter("xT", [128, NK2, NJ, 2, 512], FP8, isOutput=False)
    xres = nc.declare_dram_parameter("xres", [TLOC, C], F32, isOutput=False)
    wqk = nc.declare_dram_parameter("wqk", [128, NK2, 4, 2, 128], FP8, isOutput=False)
    wv = nc.declare_dram_parameter("wv", [128, NK2, 2, 256], FP8, isOutput=False)
    gqk = nc.declare_dram_parameter("gqk", [1, 4, 128], FP8, isOutput=False)
    gv = nc.declare_dram_parameter("gv", [1, 256], FP8, isOutput=False)
    wo = nc.declare_dram_parameter("wo", [128, 2, 2, 512], FP8, isOutput=False)
    wff = nc.declare_dram_parameter("wff", [NF, 128, NK2, 2, 256], FP8, isOutput=False)
    w3a = nc.declare_dram_parameter("w3a", [128, NF, 512], BF, isOutput=False)
    w3b = nc.declare_dram_parameter("w3b", [128, NF, 512], BF, isOutput=False)
    if have_bw:
        bwqk = nc.declare_dram_parameter("bwqk", [1, 4, 128], FP8, isOutput=False)
        bwv = nc.declare_dram_parameter("bwv", [1, 256], FP8, isOutput=False)
        bw1 = nc.declare_dram_parameter("bw1", [HIDP], F32, isOutput=False)
        bw2 = nc.declare_dram_parameter("bw2", [HIDP], F32, isOutput=False)
    out = nc.declare_dram_parameter("out", [TLOC, C], F32, isOutput=True)
    if debug:
        d_qdr = nc.declare_dram_parameter("d_qdr", [128, NJ, 2, 512], FP8, isOutput=True)
        d_kdr = nc.declare_dram_parameter("d_kdr", [128, NT, 2, 128], FP8, isOutput=True)
        d_vsb = nc.declare_dram_parameter("d_vsb", [128, HPC, NT, 128], FP8, isOutput=True)
        d_ctxT = nc.declare_dram_parameter("d_ctxT", [128, NT, 2, 128], FP8, isOutput=True)
        d_attn = nc.declare_dram_parameter("d_attn", [128, 2, 4, C], BF, isOutput=True)
        d_hn2T = nc.declare_dram_parameter("d_hn2T", [128, NKC, TLOC], BF, isOutput=True)
        d_rstd = nc.declare_dram_parameter("d_rstd", [1, T], F32, isOutput=True)
        d_negmu = nc.declare_dram_parameter("d_negmu", [1, T], FP8, isOutput=True)

    rs_in = nc.dram_tensor("rs_in", [T, C], BF)
    rstd_dram = nc.dram_tensor("rstd_dram", [T], F32)
    rs_out = nc.dram_tensor("rs_out", [TLOC, C], BF)

    mul = mybir.AluOpType.mult

    with tile.TileContext(nc) as tc:
        from contextlib import ExitStack
        with ExitStack() as ctx:
            consts = ctx.enter_context(tc.tile_pool(name="consts", bufs=1))
            ident = consts.tile([128, 128], BF)
            make_identity(nc, ident)
            maskA = consts.tile([128, 128], FP8)
            nc.gpsimd.memset(maskA, -128.0)
            # keep -128 where col(s) > row(k), else 0
            nc.gpsimd.affine_select(
                out=maskA, in_=maskA, compare_op=mybir.AluOpType.is_ge,
                fill=0.0, base=-1, pattern=[[1, 128]], channel_multiplier=-1)
            maskB = consts.tile([128, 512], FP8)
            nc.gpsimd.memset(maskB, 0.0)
            nc.gpsimd.memset(maskB[:, 0:128], 60.0)
            # keep 60 where col(t) <= row(k), else 0
            nc.gpsimd.affine_select(
                out=maskB[:, 0:128], in_=maskB[:, 0:128],
                compare_op=mybir.AluOpType.is_ge,
                fill=0.0, base=0, pattern=[[-1, 128]], channel_multiplier=1)
            epsc = consts.tile([128, 1], F32)
            nc.vector.memset(epsc, EPS)
            lnsp = consts.tile([128, 1], F32)
            nc.vector.memset(lnsp, float(np.log(SP)))
            scexp = consts.tile([128, 1], F32)
            nc.vector.memset(scexp, 1.0 / (SQ * SK))
            half_c = consts.tile([128, 1], F32)
            nc.vector.memset(half_c, 0.5)
            thsc = consts.tile([128, 1], F32)
            nc.vector.memset(thsc, 0.5 / 1024.0)
            scsq = consts.tile([128, 1], F32)
            nc.vector.memset(scsq, np.sqrt(SSQ) / SX)
            ones32 = consts.tile([128, 2, 32], FP8)
            nc.vector.memset(ones32, 1.0)
            onesbf = consts.tile([1, 64], F32)
            nc.vector.memset(onesbf, 1.0)
            onescol = consts.tile([128, 1], BF)
            nc.vector.memset(onescol, 1.0)
            gqk_sb = consts.tile([1, 4, 128], FP8)
            nc.sync.dma_start(out=gqk_sb, in_=gqk[:])
            gv_sb = consts.tile([1, 256], FP8)
            nc.sync.dma_start(out=gv_sb, in_=gv[:])
            wqk_sb = consts.tile([128, NK2, 4, 2, 128], FP8)
            nc.sync.dma_start(out=wqk_sb, in_=wqk[:])
            wv_sb = consts.tile([128, NK2, 2, 256], FP8)
            nc.sync.dma_start(out=wv_sb, in_=wv[:])
            wo_sb = consts.tile([128, 2, 2, 512], FP8)
            nc.sync.dma_start(out=wo_sb, in_=wo[:])
            if have_bw:
                bwqk_sb = consts.tile([1, 4, 128], FP8)
                nc.sync.dma_start(out=bwqk_sb, in_=bwqk[:])
                bwv_sb = consts.tile([1, 256], FP8)
                nc.sync.dma_start(out=bwv_sb, in_=bwv[:])
                onesr = consts.tile([1, 512], FP8)
                nc.vector.memset(onesr, 128.0)
                bw1c = consts.tile([128, NF], F32)
                nc.sync.dma_start(out=bw1c, in_=bw1[:].rearrange("(f p) -> p f", p=128))
                bw2c = consts.tile([128, NF], F32)
                nc.sync.dma_start(out=bw2c, in_=bw2[:].rearrange("(f p) -> p f", p=128))

            # ---------------- Phase A: stats + fused-LN QKV (fp8 DR) --------
            pA = ctx.enter_context(tc.tile_pool(name="pA", bufs=1))
            # qdr: [p=32h+d%32, J, slot=d//32, t]   kdr: [p, si, slot, t]
            qdr = pA.tile([128, NJ, 2, 512], FP8)
            kdr = pA.tile([128, NT, 2, 128], FP8)
            v_sb = pA.tile([128, HPC, NT, 128], FP8)  # [s%128, h, si, d|1|junk]
            nc.vector.memset(v_sb[:, :, :, 64:65], 1.0)
            ctxT = pA.tile([128, NT, 2, 128], FP8)  # [p=c%128, ti, dc=c//128, t]
            negmu8 = pA.tile([1, T], FP8)
            rstd_row = pA.tile([1, T], F32)
            rstd_col = pA.tile([128, 16], F32)

            with tc.tile_pool(name="xTp", bufs=1) as xTp, \
                 tc.tile_pool(name="rbp", bufs=1) as rbp, \
                 tc.tile_pool(name="xsqp", bufs=2) as xsqp, \
                 tc.tile_pool(name="strow", bufs=4) as strow, \
                 tc.tile_pool(name="psS", bufs=1, space="PSUM") as psS, \
                 tc.tile_pool(name="psA", bufs=3, space="PSUM") as psA, \
                 tc.tile_pool(name="psV", bufs=2, space="PSUM") as psV:
                xT_sb = xTp.tile([128, NK2, NJ, 2, 512], FP8)
                rstd_b = rbp.tile([128, T], F32)
                for k2 in range(NK2):
                    nc.sync.dma_start(out=xT_sb[:, k2, 0], in_=xT[:, k2, 0])

                qk_imm = [SQ / (8.0 * SX * SW), SQ / (8.0 * SX * SW),
                          SK / (SX * SW), SK / (SX * SW)]

                def emit_stats(tq):
                    xsq = xsqp.tile([128, NK2, 2, 512], FP8, tag="xsq")
                    for k2 in range(NK2):
                        # (sqrt(SSQ)/SX * SX*x)^2 = SSQ*x^2
                        nc.scalar.activation(out=xsq[:, k2], in_=xT_sb[:, k2, tq],
                                             func=AF.Square,
                                             scale=scsq[:, 0:1])
                    mu_ps = psS.tile([32, 512], F32, tag="mu")
                    sq_ps = psS.tile([32, 512], F32, tag="sq")
                    for k2 in range(NK2):
                        nc.tensor.matmul(mu_ps, ones32, xT_sb[:, k2, tq],
                                         start=(k2 == 0), stop=(k2 == NK2 - 1),
                                         perf_mode=DRM)
                        nc.tensor.matmul(sq_ps, ones32, xsq[:, k2],
                                         start=(k2 == 0), stop=(k2 == NK2 - 1),
                                         perf_mode=DRM)
                    sl = slice(tq * 512, (tq + 1) * 512)
                    mu_f = strow.tile([1, 512], F32, tag="muf")
                    nc.vector.tensor_scalar_mul(mu_f, mu_ps[0:1], 1.0 / (SX * C))
                    nc.vector.tensor_scalar_mul(negmu8[:, sl], mu_ps[0:1],
                                                -SMU / (SX * C))
                    mu2 = strow.tile([1, 512], F32, tag="mu2")
                    nc.vector.tensor_mul(mu2, mu_f, mu_f)
                    var = strow.tile([1, 512], F32, tag="var")
                    nc.vector.tensor_scalar_mul(var, sq_ps[0:1], 1.0 / (SSQ * C))
                    nc.vector.tensor_sub(var, var, mu2)
                    sd = strow.tile([1, 512], F32, tag="sd")
                    nc.scalar.activation(out=sd, in_=var, func=AF.Sqrt,
                                         bias=epsc[0:1, 0:1])
                    nc.vector.reciprocal_approx_fast(rstd_row[:, sl], sd)
                    nc.gpsimd.partition_broadcast(rstd_b[:, sl], rstd_row[:, sl])

                def emit_qk(tq):
                    sl = slice(tq * 512, (tq + 1) * 512)
                    for jb in range(4):
                        ps = psA.tile([128, 512], F32, tag="qkv")
                        for k2 in range(NK2):
                            nc.tensor.matmul(ps, wqk_sb[:, k2, jb],
                                             xT_sb[:, k2, tq],
                                             start=(k2 == 0), stop=False,
                                             perf_mode=DRM)
                        nc.tensor.matmul(ps, gqk_sb[:, jb], negmu8[0:1, sl],
                                         start=False, stop=not have_bw)
                        if have_bw:
                            nc.tensor.matmul(ps, bwqk_sb[:, jb], onesr,
                                             start=False, stop=True)
                        if jb < 2:
                            nc.vector.scalar_tensor_tensor(
                                out=qdr[:, tq, jb, :], in0=ps,
                                scalar=qk_imm[jb], in1=rstd_b[:, sl],
                                op0=mul, op1=mul)
                        else:
                            nc.vector.scalar_tensor_tensor(
                                out=kdr[:, 4 * tq:4 * tq + 4, jb - 2, :],
                                in0=ps.rearrange("p (a d) -> p a d", a=4),
                                scalar=qk_imm[jb],
                                in1=rstd_b[:, sl].rearrange(
                                    "p (a d) -> p a d", a=4),
                                op0=mul, op1=mul)

                # software pipeline: stats(tq) overlaps qk(tq-1) on the PE
                for tq in range(NJ):
                    emit_stats(tq)
                    if tq + 1 < NJ:
                        for k2 in range(NK2):
                            nc.sync.dma_start(out=xT_sb[:, k2, tq + 1],
                                              in_=xT[:, k2, tq + 1])
                    if tq >= 1:
                        emit_qk(tq - 1)
                emit_qk(NJ - 1)
                # rstd in token-partition-major layout for the v path
                nc.sync.dma_start(
                    out=rstd_dram[:].rearrange("(o t) -> o t", o=1),
                    in_=rstd_row)
                nc.sync.dma_start(
                    out=rstd_col,
                    in_=rstd_dram[:].rearrange("(ti p) -> p ti", p=128))

                # v: token-partition-major via DR matmuls on xT tiles
                for si in range(NT):
                    vps = psV.tile([128, 256], F32, tag="vps")
                    ssl = slice((si % 4) * 128, (si % 4) * 128 + 128)
                    for k2 in range(NK2):
                        nc.tensor.matmul(vps, xT_sb[:, k2, si // 4, :, ssl],
                                         wv_sb[:, k2],
                                         start=(k2 == 0), stop=False,
                                         perf_mode=DRM)
                    msl = slice(si * 128, si * 128 + 128)
                    nc.tensor.matmul(vps, negmu8[0:1, msl], gv_sb,
                                     start=False, stop=not have_bw)
                    if have_bw:
                        nc.tensor.matmul(vps, onesr[0:1, 0:128], bwv_sb,
                                         start=False, stop=True)
                    nc.vector.tensor_scalar(
                        out=v_sb[:, :, si, 0:64],
                        in0=vps.rearrange("p (h d) -> p h d", h=4),
                        scalar1=rstd_col[:, si:si + 1],
                        scalar2=SV / (SX * SW), op0=mul, op1=mul)

            # ---- attention (fp8 DR) + Wo + per-J ReduceScatter + eprep ----
            pE = ctx.enter_context(tc.tile_pool(name="pE", bufs=1))
            out1 = pE.tile([128, 4, C], F32)
            hn2T = pE.tile([128, NKC, TLOC], BF)
            hn8 = pE.tile([128, NK2, 2, TLOC], FP8)
            g_sb = pE.tile([128, NF, TLOC], BF)
            w3akeep = pE.tile([128, NF, 512], BF)
            w3keep = pE.tile([128, NF, 512], BF)
            rs_sb = pE.tile([128, 4, C], BF)
            xr_sb = pE.tile([128, 4, C], F32)
            attn_sb = pE.tile([128, 2, 4, C], BF)
            nc.sync.dma_start(out=xr_sb,
                              in_=xres[:].rearrange("(a p) c -> p a c", p=128))
            stE = ctx.enter_context(tc.tile_pool(name="stE", bufs=2))

            with tc.tile_pool(name="scps", bufs=2, space="PSUM") as scps, \
                 tc.tile_pool(name="avps", bufs=1, space="PSUM") as avps, \
                 tc.tile_pool(name="wops", bufs=1, space="PSUM") as wops, \
                 tc.tile_pool(name="band", bufs=3) as bandp, \
                 tc.tile_pool(name="rbps", bufs=1, space="PSUM") as rbps, \
                 tc.tile_pool(name="stC", bufs=2) as stC:

                def emit_attn(J):
                    nst = 4 * J + 4
                    Jsl = slice(J * 512, (J + 1) * 512)
                    for hp in range(2):
                        avs = [avps.tile([128, 512], F32, tag=f"av{h}",
                                         name=f"av{h}")
                               for h in range(2)]
                        for si2 in range(nst // 2):
                            bd2 = bandp.tile([128, 2, 2, 512], FP8, tag="bd")
                            # layout [h, sis, t] for contiguous av rhs
                            for sis in range(2):
                                si = 2 * si2 + sis
                                w = si - 4 * J
                                off = max(w, 0) * 128
                                scp = scps.tile([128, 2, 512], F32, tag="sc")
                                for hidx in range(2):
                                    h = 2 * hp + hidx
                                    nc.tensor.matmul(
                                        scp[:, hidx, off:],
                                        kdr[32 * h:32 * h + 32, si],
                                        qdr[32 * h:32 * h + 32, J, :, off:],
                                        start=True, stop=(w < 0),
                                        perf_mode=DRM,
                                        tile_position=(32 * h, 0))
                                    if w >= 0:
                                        nc.tensor.matmul(
                                            scp[:, hidx, off:],
                                            maskA, maskB[:, 0:512 - off],
                                            start=False, stop=True)
                                if w >= 0 and off > 0:
                                    nc.vector.memset(
                                        bd2[:, :, sis, 0:off], 0.0)
                                nc.scalar.activation(
                                    out=bd2[:, :, sis, off:],
                                    in_=scp[:, :, off:],
                                    func=AF.Exp, scale=scexp[:, 0:1],
                                    bias=lnsp[:, 0:1])
                            for hidx in range(2):
                                h = 2 * hp + hidx
                                nc.tensor.matmul(
                                    avs[hidx], v_sb[:, h, 2 * si2:2 * si2 + 2, :],
                                    bd2[:, hidx], start=(si2 == 0),
                                    stop=(si2 == nst // 2 - 1), perf_mode=DRM)
                        for hidx in range(2):
                            h = 2 * hp + hidx
                            po, hs = (h % 2) * 64, h // 2
                            dnm = stC.tile([1, 512], F32, tag="dnm")
                            nc.vector.tensor_copy(dnm, avs[hidx][64:65, :])
                            rrow = stC.tile([1, 512], F32, tag="rr")
                            nc.vector.reciprocal_approx_fast(rrow, dnm)
                            rb_ps = rbps.tile([64, 512], F32, tag="rbp")
                            nc.tensor.matmul(rb_ps, onesbf, rrow,
                                             start=True, stop=True)
                            rb = stC.tile([64, 512], F32, tag="rb")
                            nc.vector.tensor_copy(rb, rb_ps)
                            nc.vector.tensor_mul(
                                ctxT[po:po + 64, 4 * J:4 * J + 4, hs, :],
                                avs[hidx][0:64, :].rearrange(
                                    "p (a d) -> p a d", a=4),
                                rb.rearrange("p (a d) -> p a d", a=4))
                    # Wo for this J's 4 token tiles
                    for ti in range(4 * J, 4 * J + 4):
                        for nh in range(2):
                            wp = wops.tile([128, 512], F32, tag="wp")
                            nc.tensor.matmul(wp, ctxT[:, ti], wo_sb[:, nh],
                                             start=True, stop=True,
                                             perf_mode=DRM)
                            nc.vector.tensor_scalar_mul(
                                attn_sb[:, J % 2, ti - 4 * J,
                                        nh * 512:(nh + 1) * 512],
                                wp, 1.0 / (SV * SWO))
                    t0 = J * 512
                    nc.sync.dma_start(
                        out=rs_in[t0:t0 + 512, :].rearrange(
                            "(a p) c -> p a c", p=128),
                        in_=attn_sb[:, J % 2])
                    if sim:
                        nc.sync.dma_start(out=rs_out[J * 128:(J + 1) * 128, :],
                                          in_=rs_in[t0:t0 + 128, :])
                    else:
                        nc.gpsimd.collective_compute(
                            "ReduceScatter", mybir.AluOpType.add,
                            replica_groups=RG,
                            ins=[rs_in[t0:t0 + 512, :]],
                            outs=[rs_out[J * 128:(J + 1) * 128, :]])

                def emit_eprep(J):
                    # LN2 of (x + attn) for owned chunk J -> hn2T
                    nc.sync.dma_start(
                        out=rs_sb[:, J, :],
                        in_=rs_out[J * 128:(J + 1) * 128, :].rearrange(
                            "(a p) c -> p a c", p=128))
                    o1 = out1[:, J, :]
                    nc.vector.tensor_add(o1, xr_sb[:, J, :], rs_sb[:, J, :])
                    st = stE.tile([128, 2, 6], F32, tag="st")
                    nc.vector.bn_stats(out=st[:, 0, :], in_=o1[:, 0:512])
                    nc.vector.bn_stats(out=st[:, 1, :], in_=o1[:, 512:1024])
                    mv = stE.tile([128, 2], F32, tag="mv")
                    nc.vector.bn_aggr(out=mv, in_=st)
                    ve = stE.tile([128, 1], F32, tag="ve")
                    nc.vector.tensor_scalar_add(ve, mv[:, 1:2], EPS)
                    # Newton rsqrt from seed 1.0 (var of x+attn is near 1)
                    rstd = stE.tile([128, 1], F32, tag="rstd")
                    nc.vector.memset(rstd, 1.0)
                    nwt = stE.tile([128, 1], F32, tag="nwt")
                    for _ in range(3):
                        nc.vector.tensor_mul(nwt, rstd, rstd)
                        nc.vector.tensor_mul(nwt, nwt, ve)
                        nc.vector.tensor_scalar(nwt, nwt, -0.5, 1.5, mul,
                                                mybir.AluOpType.add)
                        nc.vector.tensor_mul(rstd, rstd, nwt)
                    rmu = stE.tile([128, 1], F32, tag="rmu")
                    nc.vector.tensor_mul(rmu, mv[:, 0:1], rstd)
                    hn2 = stE.tile([128, C], BF, tag="hn2")
                    nc.vector.tensor_scalar(hn2, o1, rstd, rmu, mul,
                                            mybir.AluOpType.subtract)
                    nc.sync.dma_start_transpose(
                        hn2T[:, :, J * 128:(J + 1) * 128], hn2)
                    nc.vector.tensor_scalar_mul(
                        hn8[:, :, :, J * 128:(J + 1) * 128],
                        hn2T[:, :, J * 128:(J + 1) * 128].rearrange(
                            "p (a b) t -> p a b t", b=2), 16.0)

                emit_attn(0)
                emit_attn(1)
                emit_attn(2)
                emit_eprep(0)
                emit_eprep(1)
                emit_attn(3)
                emit_eprep(2)

            # ---------------- FFN: SwiGLU (bf16, tanh-silu) ----------
            with tc.tile_pool(name="wstream", bufs=6) as wstream, \
                 tc.tile_pool(name="gtmp", bufs=3) as gtmp, \
                 tc.tile_pool(name="psG", bufs=3, space="PSUM") as psG, \
                 tc.tile_pool(name="psW3", bufs=1, space="PSUM") as psW3:
                nc.sync.dma_start(out=w3akeep, in_=w3a[:])
                nc.sync.dma_start(out=w3keep, in_=w3b[:])

                def emit_z(half):
                    hsl = slice(half * 256, half * 256 + 256)
                    for fi in range(NF):
                        wt = wstream.tile([128, NK2, 2, 256], FP8, tag="wt")
                        nc.sync.dma_start(out=wt, in_=wff[fi])
                        z1 = psG.tile([128, 256], F32, tag="z1")
                        for k2 in range(NK2):
                            nc.tensor.matmul(z1, wt[:, k2, :, 0:128],
                                             hn8[:, k2, :, hsl],
                                             start=(k2 == 0),
                                             stop=(k2 == NK2 - 1),
                                             perf_mode=DRM)
                        th = gtmp.tile([128, 256], BF, tag="th")
                        nc.scalar.activation(out=th, in_=z1, func=AF.Tanh,
                                             scale=thsc[:, 0:1])
                        sil = gtmp.tile([128, 256], BF, tag="sil")
                        nc.vector.scalar_tensor_tensor(
                            out=sil, in0=th, scalar=1.0, in1=z1,
                            op0=mybir.AluOpType.add, op1=mul)
                        z2 = psG.tile([128, 256], F32, tag="z1")
                        for k2 in range(NK2):
                            nc.tensor.matmul(z2, wt[:, k2, :, 128:256],
                                             hn8[:, k2, :, hsl],
                                             start=(k2 == 0),
                                             stop=(k2 == NK2 - 1),
                                             perf_mode=DRM)
                        nc.vector.scalar_tensor_tensor(
                            out=g_sb[:, fi, hsl], in0=sil,
                            scalar=1.0 / 1048576.0, in1=z2, op0=mul, op1=mul)

                def emit_acc(tts):
                    for cw, w3t in ((0, w3akeep), (1, w3keep)):
                        acc = psW3.tile([128, 2, 512], F32, tag="acc",
                                        name="acc")
                        for fi in range(NF):
                            for i, tt in enumerate(tts):
                                nc.tensor.matmul(
                                    acc[:, i, :],
                                    g_sb[:, fi, tt * 128:(tt + 1) * 128],
                                    w3t[:, fi, :],
                                    start=(fi == 0), stop=(fi == NF - 1))
                        for i, tt in enumerate(tts):
                            csl = slice(cw * 512, cw * 512 + 512)
                            nc.vector.tensor_add(xr_sb[:, tt, csl],
                                                 acc[:, i, :], out1[:, tt, csl])
                    for tt in tts:
                        nc.sync.dma_start(
                            out=out[128 * tt:128 * (tt + 1), :].rearrange(
                                "(a p) c -> p a c", a=1),
                            in_=xr_sb[:, tt:tt + 1, :])

                emit_z(0)
                emit_acc([0, 1])
                emit_eprep(3)
                emit_z(1)
                emit_acc([2, 3])
    nc.compile()
    return nc


def _prep(x, Wq, Wk, Wv, Wo, W1, W2, W3, gamma, beta):
    f32 = np.float32
    gcol = gamma.astype(f32)[:, None]
    have_bw = bool(np.any(beta != 0))
    in_maps = []
    for c in range(8):
        b, r = c // 4, c % 4
        hh = [4 * r + h for h in range(HPC)]
        # q/k columns permuted for DR attention layout: col = 32h + (d % 32),
        # jb blocks: (q,slot0),(q,slot1),(k,slot0),(k,slot1)
        wqk_cols = np.empty((C, 4, 128), f32)
        for jbi, (W, slot) in enumerate(
                [(Wq, 0), (Wq, 1), (Wk, 0), (Wk, 1)]):
            for hi, h in enumerate(hh):
                wqk_cols[:, jbi, 32 * hi:32 * hi + 32] = \
                    W[h][:, 32 * slot:32 * slot + 32]
        wqk8 = (gcol[:, None] * wqk_cols * SW).astype(E4)
        # v columns: plain head-major 64h+d
        wv_cols = np.concatenate([Wv[h] for h in hh], axis=1).astype(f32)
        wv8 = (gcol * wv_cols * SW).astype(E4)
        # correction rows from the rounded weights
        gqk8 = (wqk8.astype(f32).sum(axis=0) * (SGWS / SW)).astype(E4)[None]
        gv8 = (wv8.astype(f32).sum(axis=0) * (SGWS / SW)).astype(E4)[None]
        # DR contraction packing [p, k2, ...]: c-dim index = 256*k2+128*slot+p
        wqk_dr = np.ascontiguousarray(
            wqk8.reshape(NK2, 2, 128, 4, 128).transpose(2, 0, 3, 1, 4))
        wv_dr = np.ascontiguousarray(
            wv8.reshape(NK2, 2, 128, 256).transpose(2, 0, 1, 3))
        # Wo rows: local row = 64*hi+d = 128*slot+p
        wo_loc = (Wo[r * 256:(r + 1) * 256, :].astype(f32) * SWO).astype(E4)
        wo_dr = np.ascontiguousarray(
            wo_loc.reshape(2, 128, 2, 512).transpose(1, 2, 0, 3))
        # FFN (bf16, unchanged math)
        w1p = np.zeros((C, HIDP), f32)
        w1p[:, :HID] = W1
        w2p = np.zeros((C, HIDP), f32)
        w2p[:, :HID] = W2
        w3p = np.zeros((HIDP, C), f32)
        w3p[:HID, :] = W3
        w1g = (gcol * w1p).reshape(NKC, 128, NF, 128).transpose(2, 1, 0, 3).reshape(NF, 128, C)
        w2g = (gcol * w2p).reshape(NKC, 128, NF, 128).transpose(2, 1, 0, 3).reshape(NF, 128, C)
        w3r = w3p.reshape(NF, 128, C)
        # [NF, 128p, k2, slot, (w1 128 | 0.5*w2 128)] fp8, x64
        # w1g[fi] dims: [c%128 partition, (kc, f) flattened]
        wff = np.empty((NF, 128, NK2, 2, 256), np.float32)
        for fi in range(NF):
            for proj, wsrc in ((0, w1g), (1, 0.5 * w2g)):
                # [c%128, kc, f] -> group kc into (k2, slot)
                wr = wsrc[fi].reshape(128, NK2, 2, 128) * SW
                wff[fi, :, :, :, proj * 128:(proj + 1) * 128] = wr
        w3ab = np.ascontiguousarray(
            w3r[:, :, 0:512].transpose(1, 0, 2)).astype(BF16)
        w3bb = np.ascontiguousarray(
            w3r[:, :, 512:1024].transpose(1, 0, 2)).astype(BF16)
        xb = x[b].astype(f32)
        # x^T fp8 in DR layout [p, k2, tq, slot, 512]
        xs = np.ascontiguousarray(xb.T * SX).astype(E4)  # [C, T]
        xT_dr = np.ascontiguousarray(
            xs.reshape(NK2, 2, 128, NJ, 512).transpose(2, 0, 3, 1, 4))
        xres = np.concatenate(
            [xb[512 * J + 128 * r: 512 * J + 128 * (r + 1)] for J in range(NJ)])
        m = {
            "xT": xT_dr,
            "xres": np.ascontiguousarray(xres),
            "wqk": wqk_dr,
            "wv": wv_dr,
            "gqk": np.ascontiguousarray(gqk8.reshape(1, 4, 128)),
            "gv": gv8,
            "wo": wo_dr,
            "wff": wff.astype(E4),
            "w3a": w3ab,
            "w3b": w3bb,
        }
        if have_bw:
            bqk = (beta.astype(f32) @ wqk_cols.reshape(C, 512)) * 8.0
            m["bwqk"] = bqk.astype(E4).reshape(1, 4, 128)
            m["bwv"] = ((beta.astype(f32) @ wv_cols) * 8.0).astype(E4)[None]
            m["bw1"] = (beta.astype(f32) @ w1p).astype(f32)
            m["bw2"] = (beta.astype(f32) @ w2p).astype(f32)
        in_maps.append(m)
    return in_maps, have_bw


def kernel(x, Wq, Wk, Wv, Wo, W1, W2, W3, gamma, beta, _bench=None,
           _debug=False):
    x = np.asarray(x)
    in_maps, have_bw = _prep(np.asarray(x), np.asarray(Wq), np.asarray(Wk),
                             np.asarray(Wv), np.asarray(Wo), np.asarray(W1),
                             np.asarray(W2), np.asarray(W3),
                             np.asarray(gamma), np.asarray(beta))
    key = ("k", have_bw, _debug)
    if key not in _cache:
        _cache[key] = _build(have_bw, debug=_debug)
    nc = _cache[key]
    kw = dict(_bench) if _bench else {}
    res = run_bass_kernel_spmd(nc, in_maps, list(range(8)), **kw)
    outf = np.empty((B, T, C), np.float32)
    for c in range(8):
        b, r = c // 4, c % 4
        o = res.results[c]["out"]
        for J in range(NJ):
            outf[b, 512 * J + 128 * r: 512 * J + 128 * (r + 1)] = \
                o[128 * J:128 * (J + 1)]
    if _bench is not None:
        kernel.last_results = res
    global _last_res
    _last_res = res
    return outf


# revision 27
# speedup vs baseline: 1.0229x; 1.0229x over previous
"""Trainium2 Bass kernel for a pre-LN causal decoder layer (MHA + SwiGLU).

Sharding: 2-way data parallel over batch x 4-way tensor parallel over heads.
Core c (of 8): batch b=c//4, group rank r=c%4, heads [4r, 4r+4).

fp8(e4m3) DoubleRow matmuls for QKV projections, QK^T, AV, and Wo (2-4x PE
throughput vs bf16); SwiGLU FFN stays bf16 for accuracy. LayerNorm is folded
into the projections: matmuls run on raw 16*x (fp8) and the per-token
(mean, rstd) fixup is applied at the PSUM->SBUF quantize stage; gamma and all
fp8 scale factors are folded into the weights on the host.

Attention emits one ReduceScatter chunk per 512-token J-block (4 chunks), so
the FFN's input (LN2 of attn+residual) is mostly ready when attention ends.
"""

import sys

sys.path.insert(0, "/opt/trn_rl_repo")

import numpy as np
import ml_dtypes

import concourse.bass as bass
import concourse.mybir as mybir
import concourse.tile as tile
from concourse import bacc
from concourse.bass_utils import run_bass_kernel_spmd
from concourse.masks import make_identity

BF16 = ml_dtypes.bfloat16
E4 = ml_dtypes.float8_e4m3
F32 = mybir.dt.float32
BF = mybir.dt.bfloat16
FP8 = mybir.dt.float8e4
DRM = mybir.MatmulPerfMode.DoubleRow
AF = mybir.ActivationFunctionType

B, T, C = 2, 2048, 1024
H, HS = 16, 64
HID = 2730
HIDP = 2816  # padded to 22*128
NF = HIDP // 128  # 22
HPC = 4  # heads per core
TLOC = T // 4  # 512 tokens owned post-RS
EPS = 1e-3
RG = [[0, 1, 2, 3], [4, 5, 6, 7]]
NT = T // 128  # 16 s-tiles
NJ = T // 512  # 4 J-blocks
NKC = C // 128  # 8 contraction chunks
NK2 = NKC // 2  # 4 DR contraction pairs

# fp8 scale factors
SX = 16.0    # x
SW = 64.0    # qkv weights
SMU = 512.0  # negmu
SGWS = SX * SW / SMU  # = 2; gws correction row
SSQ = 4.0    # x^2 (4*x^2 max ~92 < e4m3 max 224)
SQ = 32.0    # q (with 1/8 folded)
SK = 4.0     # k
SV = 4.0     # v
SP = 8.0     # exp probs
SWO = 64.0   # Wo
NEGF = -60.0 * SQ * SK  # mask fill in scaled-score units

_cache = {}


def _build(have_bw, sim=False, debug=False):
    nc = bacc.Bacc(None, target_bir_lowering=False, debug=False)
    xT = nc.declare_dram_parameter("xT", [128, NK2, NJ, 2, 512], FP8, isOutput=False)
    xres = nc.declare_dram_parameter("xres", [TLOC, C], F32, isOutput=False)
    wqk = nc.declare_dram_parameter("wqk", [128, NK2, 4, 2, 128], FP8, isOutput=False)
    wv = nc.declare_dram_parameter("wv", [128, NK2, 2, 256], FP8, isOutput=False)
    gqk = nc.declare_dram_parameter("gqk", [1, 4, 128], FP8, isOutput=False)
    gv = nc.declare_dram_parameter("gv", [1, 256], FP8, isOutput=False)
    wo = nc.declare_dram_parameter("wo", [128, 2, 2, 512], FP8, isOutput=False)
    wff = nc.declare_dram_parameter("wff", [NF, 128, NK2, 2, 256], FP8, isOutput=False)
    w3a = nc.declare_dram_parameter("w3a", [128, NF, 512], BF, isOutput=False)
    w3b = nc.declare_dram_parameter("w3b", [128, NF, 512], BF, isOutput=False)
    if have_bw:
        bwqk = nc.declare_dram_parameter("bwqk", [1, 4, 128], FP8, isOutput=False)
        bwv = nc.declare_dram_parameter("bwv", [1, 256], FP8, isOutput=False)
        bw1 = nc.declare_dram_parameter("bw1", [HIDP], F32, isOutput=False)
        bw2 = nc.declare_dram_parameter("bw2", [HIDP], F32, isOutput=False)
    out = nc.declare_dram_parameter("out", [TLOC, C], F32, isOutput=True)
    if debug:
        d_qdr = nc.declare_dram_parameter("d_qdr", [128, NJ, 2, 512], FP8, isOutput=True)
        d_kdr = nc.declare_dram_parameter("d_kdr", [128, NT, 2, 128], FP8, isOutput=True)
        d_vsb = nc.declare_dram_parameter("d_vsb", [128, HPC, NT, 128], FP8, isOutput=True)
        d_ctxT = nc.declare_dram_parameter("d_ctxT", [128, NT, 2, 128], FP8, isOutput=True)
        d_attn = nc.declare_dram_parameter("d_attn", [128, 2, 4, C], BF, isOutput=True)
        d_hn2T = nc.declare_dram_parameter("d_hn2T", [128, NKC, TLOC], BF, isOutput=True)
        d_rstd = nc.declare_dram_parameter("d_rstd", [1, T], F32, isOutput=True)
        d_negmu = nc.declare_dram_parameter("d_negmu", [1, T], FP8, isOutput=True)

    rs_in = nc.dram_tensor("rs_in", [T, C], BF)
    rstd_dram = nc.dram_tensor("rstd_dram", [T], F32)
    rs_out = nc.dram_tensor("rs_out", [TLOC, C], BF)

    mul = mybir.AluOpType.mult

    with tile.TileContext(nc) as tc:
        from contextlib import ExitStack
        with ExitStack() as ctx:
            consts = ctx.enter_context(tc.tile_pool(name="consts", bufs=1))
            ident = consts.tile([128, 128], BF)
            make_identity(nc, ident)
            maskA = consts.tile([128, 128], FP8)
            nc.gpsimd.memset(maskA, -128.0)
            # keep -128 where col(s) > row(k), else 0
            nc.gpsimd.affine_select(
                out=maskA, in_=maskA, compare_op=mybir.AluOpType.is_ge,
                fill=0.0, base=-1, pattern=[[1, 128]], channel_multiplier=-1)
            maskB = consts.tile([128, 512], FP8)
            nc.gpsimd.memset(maskB, 0.0)
            nc.gpsimd.memset(maskB[:, 0:128], 60.0)
            # keep 60 where col(t) <= row(k), else 0
            nc.gpsimd.affine_select(
                out=maskB[:, 0:128], in_=maskB[:, 0:128],
                compare_op=mybir.AluOpType.is_ge,
                fill=0.0, base=0, pattern=[[-1, 128]], channel_multiplier=1)
            epsc = consts.tile([128, 1], F32)
            nc.vector.memset(epsc, EPS)
            lnsp = consts.tile([128, 1], F32)
            nc.vector.memset(lnsp, float(np.log(SP)))
            scexp = consts.tile([128, 1], F32)
            nc.vector.memset(scexp, 1.0 / (SQ * SK))
            half_c = consts.tile([128, 1], F32)
            nc.vector.memset(half_c, 0.5)
            thsc = consts.tile([128, 1], F32)
            nc.vector.memset(thsc, 0.5 / 1024.0)
            scsq = consts.tile([128, 1], F32)
            nc.vector.memset(scsq, np.sqrt(SSQ) / SX)
            ones32 = consts.tile([128, 2, 32], FP8)
            nc.vector.memset(ones32, 1.0)
            onesbf = consts.tile([1, 64], F32)
            nc.vector.memset(onesbf, 1.0)
            onescol = consts.tile([128, 1], BF)
            nc.vector.memset(onescol, 1.0)
            gqk_sb = consts.tile([1, 4, 128], FP8)
            nc.sync.dma_start(out=gqk_sb, in_=gqk[:])
            gv_sb = consts.tile([1, 256], FP8)
            nc.sync.dma_start(out=gv_sb, in_=gv[:])
            wqk_sb = consts.tile([128, NK2, 4, 2, 128], FP8)
            nc.sync.dma_start(out=wqk_sb, in_=wqk[:])
            wv_sb = consts.tile([128, NK2, 2, 256], FP8)
            nc.sync.dma_start(out=wv_sb, in_=wv[:])
            wo_sb = consts.tile([128, 2, 2, 512], FP8)
            nc.sync.dma_start(out=wo_sb, in_=wo[:])
            if have_bw:
                bwqk_sb = consts.tile([1, 4, 128], FP8)
                nc.sync.dma_start(out=bwqk_sb, in_=bwqk[:])
                bwv_sb = consts.tile([1, 256], FP8)
                nc.sync.dma_start(out=bwv_sb, in_=bwv[:])
                onesr = consts.tile([1, 512], FP8)
                nc.vector.memset(onesr, 128.0)
                bw1c = consts.tile([128, NF], F32)
                nc.sync.dma_start(out=bw1c, in_=bw1[:].rearrange("(f p) -> p f", p=128))
                bw2c = consts.tile([128, NF], F32)
                nc.sync.dma_start(out=bw2c, in_=bw2[:].rearrange("(f p) -> p f", p=128))

            # ---------------- Phase A: stats + fused-LN QKV (fp8 DR) --------
            pA = ctx.enter_context(tc.tile_pool(name="pA", bufs=1))
            # qdr: [p=32h+d%32, J, slot=d//32, t]   kdr: [p, si, slot, t]
            qdr = pA.tile([128, NJ, 2, 512], FP8)
            kdr = pA.tile([128, NT, 2, 128], FP8)
            v_sb = pA.tile([128, HPC, NT, 128], FP8)  # [s%128, h, si, d|1|junk]
            nc.vector.memset(v_sb[:, :, :, 64:65], 1.0)
            ctxT = pA.tile([128, NT, 2, 128], FP8)  # [p=c%128, ti, dc=c//128, t]
            negmu8 = pA.tile([1, T], FP8)
            rstd_row = pA.tile([1, T], F32)
            rstd_col = pA.tile([128, 16], F32)

            with tc.tile_pool(name="xTp", bufs=1) as xTp, \
                 tc.tile_pool(name="rbp", bufs=1) as rbp, \
                 tc.tile_pool(name="xsqp", bufs=2) as xsqp, \
                 tc.tile_pool(name="strow", bufs=4) as strow, \
                 tc.tile_pool(name="psS", bufs=1, space="PSUM") as psS, \
                 tc.tile_pool(name="psA", bufs=3, space="PSUM") as psA, \
                 tc.tile_pool(name="psV", bufs=2, space="PSUM") as psV:
                xT_sb = xTp.tile([128, NK2, NJ, 2, 512], FP8)
                rstd_b = rbp.tile([128, T], F32)
                for k2 in range(NK2):
                    nc.sync.dma_start(out=xT_sb[:, k2, 0], in_=xT[:, k2, 0])

                qk_imm = [SQ / (8.0 * SX * SW), SQ / (8.0 * SX * SW),
                          SK / (SX * SW), SK / (SX * SW)]

                def emit_stats(tq):
                    xsq = xsqp.tile([128, NK2, 2, 512], FP8, tag="xsq")
                    for k2 in range(NK2):
                        # (sqrt(SSQ)/SX * SX*x)^2 = SSQ*x^2
                        nc.scalar.activation(out=xsq[:, k2], in_=xT_sb[:, k2, tq],
                                             func=AF.Square,
                                             scale=scsq[:, 0:1])
                    mu_ps = psS.tile([32, 512], F32, tag="mu")
                    sq_ps = psS.tile([32, 512], F32, tag="sq")
                    for k2 in range(NK2):
                        nc.tensor.matmul(mu_ps, ones32, xT_sb[:, k2, tq],
                                         start=(k2 == 0), stop=(k2 == NK2 - 1),
                                         perf_mode=DRM)
                        nc.tensor.matmul(sq_ps, ones32, xsq[:, k2],
                                         start=(k2 == 0), stop=(k2 == NK2 - 1),
                                         perf_mode=DRM)
                    sl = slice(tq * 512, (tq + 1) * 512)
                    mu_f = strow.tile([1, 512], F32, tag="muf")
                    nc.vector.tensor_scalar_mul(mu_f, mu_ps[0:1], 1.0 / (SX * C))
                    nc.vector.tensor_scalar_mul(negmu8[:, sl], mu_ps[0:1],
                                                -SMU / (SX * C))
                    mu2 = strow.tile([1, 512], F32, tag="mu2")
                    nc.vector.tensor_mul(mu2, mu_f, mu_f)
                    var = strow.tile([1, 512], F32, tag="var")
                    nc.vector.tensor_scalar_mul(var, sq_ps[0:1], 1.0 / (SSQ * C))
                    nc.vector.tensor_sub(var, var, mu2)
                    sd = strow.tile([1, 512], F32, tag="sd")
                    nc.scalar.activation(out=sd, in_=var, func=AF.Sqrt,
                                         bias=epsc[0:1, 0:1])
                    nc.vector.reciprocal_approx_fast(rstd_row[:, sl], sd)
                    nc.gpsimd.partition_broadcast(rstd_b[:, sl], rstd_row[:, sl])

                def emit_qk(tq):
                    sl = slice(tq * 512, (tq + 1) * 512)
                    for jb in range(4):
                        ps = psA.tile([128, 512], F32, tag="qkv")
                        for k2 in range(NK2):
                            nc.tensor.matmul(ps, wqk_sb[:, k2, jb],
                                             xT_sb[:, k2, tq],
                                             start=(k2 == 0), stop=False,
                                             perf_mode=DRM)
                        nc.tensor.matmul(ps, gqk_sb[:, jb], negmu8[0:1, sl],
                                         start=False, stop=not have_bw)
                        if have_bw:
                            nc.tensor.matmul(ps, bwqk_sb[:, jb], onesr,
                                             start=False, stop=True)
                        if jb < 2:
                            nc.vector.scalar_tensor_tensor(
                                out=qdr[:, tq, jb, :], in0=ps,
                                scalar=qk_imm[jb], in1=rstd_b[:, sl],
                                op0=mul, op1=mul)
                        else:
                            nc.vector.scalar_tensor_tensor(
                                out=kdr[:, 4 * tq:4 * tq + 4, jb - 2, :],
                                in0=ps.rearrange("p (a d) -> p a d", a=4),
                                scalar=qk_imm[jb],
                                in1=rstd_b[:, sl].rearrange(
                                    "p (a d) -> p a d", a=4),
                                op0=mul, op1=mul)

                # software pipeline: stats(tq) overlaps qk(tq-1) on the PE
                for tq in range(NJ):
                    emit_stats(tq)
                    if tq + 1 < NJ:
                        for k2 in range(NK2):
                            nc.sync.dma_start(out=xT_sb[:, k2, tq + 1],
                                              in_=xT[:, k2, tq + 1])
                    if tq >= 1:
                        emit_qk(tq - 1)
                emit_qk(NJ - 1)
                # rstd in token-partition-major layout for the v path
                nc.sync.dma_start(
                    out=rstd_dram[:].rearrange("(o t) -> o t", o=1),
                    in_=rstd_row)
                nc.sync.dma_start(
                    out=rstd_col,
                    in_=rstd_dram[:].rearrange("(ti p) -> p ti", p=128))

                # v: token-partition-major via DR matmuls on xT tiles
                for si in range(NT):
                    vps = psV.tile([128, 256], F32, tag="vps")
                    ssl = slice((si % 4) * 128, (si % 4) * 128 + 128)
                    for k2 in range(NK2):
                        nc.tensor.matmul(vps, xT_sb[:, k2, si // 4, :, ssl],
                                         wv_sb[:, k2],
                                         start=(k2 == 0), stop=False,
                                         perf_mode=DRM)
                    msl = slice(si * 128, si * 128 + 128)
                    nc.tensor.matmul(vps, negmu8[0:1, msl], gv_sb,
                                     start=False, stop=not have_bw)
                    if have_bw:
                        nc.tensor.matmul(vps, onesr[0:1, 0:128], bwv_sb,
                                         start=False, stop=True)
                    nc.vector.tensor_scalar(
                        out=v_sb[:, :, si, 0:64],
                        in0=vps.rearrange("p (h d) -> p h d", h=4),
                        scalar1=rstd_col[:, si:si + 1],
                        scalar2=SV / (SX * SW), op0=mul, op1=mul)

            # ---- attention (fp8 DR) + Wo + per-J ReduceScatter + eprep ----
            pE = ctx.enter_context(tc.tile_pool(name="pE", bufs=1))
            out1 = pE.tile([128, 4, C], F32)
            hn2T = pE.tile([128, NKC, TLOC], BF)
            hn8 = pE.tile([128, NK2, 2, TLOC], FP8)
            g_sb = pE.tile([128, NF, TLOC], BF)
            w3akeep = pE.tile([128, NF, 512], BF)
            w3keep = pE.tile([128, NF, 512], BF)
            rs_sb = pE.tile([128, 4, C], BF)
            xr_sb = pE.tile([128, 4, C], F32)
            attn_sb = pE.tile([128, 2, 4, C], BF)
            nc.sync.dma_start(out=xr_sb,
                              in_=xres[:].rearrange("(a p) c -> p a c", p=128))
            stE = ctx.enter_context(tc.tile_pool(name="stE", bufs=2))

            with tc.tile_pool(name="scps", bufs=2, space="PSUM") as scps, \
                 tc.tile_pool(name="avps", bufs=1, space="PSUM") as avps, \
                 tc.tile_pool(name="wops", bufs=1, space="PSUM") as wops, \
                 tc.tile_pool(name="band", bufs=3) as bandp, \
                 tc.tile_pool(name="rbps", bufs=1, space="PSUM") as rbps, \
                 tc.tile_pool(name="stC", bufs=2) as stC:

                def emit_attn(J):
                    nst = 4 * J + 4
                    Jsl = slice(J * 512, (J + 1) * 512)
                    for hp in range(2):
                        avs = [avps.tile([128, 512], F32, tag=f"av{h}",
                                         name=f"av{h}")
                               for h in range(2)]
                        for si2 in range(nst // 2):
                            bd2 = bandp.tile([128, 2, 2, 512], FP8, tag="bd")
                            # layout [h, sis, t] for contiguous av rhs
                            for sis in range(2):
                                si = 2 * si2 + sis
                                w = si - 4 * J
                                off = max(w, 0) * 128
                                scp = scps.tile([128, 2, 512], F32, tag="sc")
                                for hidx in range(2):
                                    h = 2 * hp + hidx
                                    nc.tensor.matmul(
                                        scp[:, hidx, off:],
                                        kdr[32 * h:32 * h + 32, si],
                                        qdr[32 * h:32 * h + 32, J, :, off:],
                                        start=True, stop=(w < 0),
                                        perf_mode=DRM,
                                        tile_position=(32 * h, 0))
                                    if w >= 0:
                                        nc.tensor.matmul(
                                            scp[:, hidx, off:],
                                            maskA, maskB[:, 0:512 - off],
                                            start=False, stop=True)
                                if w >= 0 and off > 0:
                                    nc.vector.memset(
                                        bd2[:, :, sis, 0:off], 0.0)
                                nc.scalar.activation(
                                    out=bd2[:, :, sis, off:],
                                    in_=scp[:, :, off:],
                                    func=AF.Exp, scale=scexp[:, 0:1],
                                    bias=lnsp[:, 0:1])
                            for hidx in range(2):
                                h = 2 * hp + hidx
                                nc.tensor.matmul(
                                    avs[hidx], v_sb[:, h, 2 * si2:2 * si2 + 2, :],
                                    bd2[:, hidx], start=(si2 == 0),
                                    stop=(si2 == nst // 2 - 1), perf_mode=DRM)
                        for hidx in range(2):
                            h = 2 * hp + hidx
                            po, hs = (h % 2) * 64, h // 2
                            dnm = stC.tile([1, 512], F32, tag="dnm")
                            nc.vector.tensor_copy(dnm, avs[hidx][64:65, :])
                            rrow = stC.tile([1, 512], F32, tag="rr")
                            nc.vector.reciprocal_approx_fast(rrow, dnm)
                            rb_ps = rbps.tile([64, 512], F32, tag="rbp")
                            nc.tensor.matmul(rb_ps, onesbf, rrow,
                                             start=True, stop=True)
                            rb = stC.tile([64, 512], F32, tag="rb")
                            nc.vector.tensor_copy(rb, rb_ps)
                            nc.vector.tensor_mul(
                                ctxT[po:po + 64, 4 * J:4 * J + 4, hs, :],
                                avs[hidx][0:64, :].rearrange(
                                    "p (a d) -> p a d", a=4),
                                rb.rearrange("p (a d) -> p a d", a=4))
                    # Wo for this J's 4 token tiles
                    for ti in range(4 * J, 4 * J + 4):
                        for nh in range(2):
                            wp = wops.tile([128, 512], F32, tag="wp")
                            nc.tensor.matmul(wp, ctxT[:, ti], wo_sb[:, nh],
                                             start=True, stop=True,
                                             perf_mode=DRM)
                            nc.vector.tensor_scalar_mul(
                                attn_sb[:, J % 2, ti - 4 * J,
                                        nh * 512:(nh + 1) * 512],
                                wp, 1.0 / (SV * SWO))
                    t0 = J * 512
                    nc.sync.dma_start(
                        out=rs_in[t0:t0 + 512, :].rearrange(
                            "(a p) c -> p a c", p=128),
                        in_=attn_sb[:, J % 2])
                    if sim:
                        nc.sync.dma_start(out=rs_out[J * 128:(J + 1) * 128, :],
                                          in_=rs_in[t0:t0 + 128, :])
                    else:
                        nc.gpsimd.collective_compute(
                            "ReduceScatter", mybir.AluOpType.add,
                            replica_groups=RG,
                            ins=[rs_in[t0:t0 + 512, :]],
                            outs=[rs_out[J * 128:(J + 1) * 128, :]])
                    nc.gpsimd.dma_start(
                        out=rs_sb[:, J, :],
                        in_=rs_out[J * 128:(J + 1) * 128, :].rearrange(
                            "(a p) c -> p a c", p=128))

                def emit_eprep(J):
                    # LN2 of (x + attn) for owned chunk J -> hn2T
                    o1 = out1[:, J, :]
                    nc.vector.tensor_add(o1, xr_sb[:, J, :], rs_sb[:, J, :])
                    st = stE.tile([128, 2, 6], F32, tag="st")
                    nc.vector.bn_stats(out=st[:, 0, :], in_=o1[:, 0:512])
                    nc.vector.bn_stats(out=st[:, 1, :], in_=o1[:, 512:1024])
                    mv = stE.tile([128, 2], F32, tag="mv")
                    nc.vector.bn_aggr(out=mv, in_=st)
                    ve = stE.tile([128, 1], F32, tag="ve")
                    nc.vector.tensor_scalar_add(ve, mv[:, 1:2], EPS)
                    # Newton rsqrt from seed 1.0 (var of x+attn is near 1)
                    rstd = stE.tile([128, 1], F32, tag="rstd")
                    nc.vector.memset(rstd, 1.0)
                    nwt = stE.tile([128, 1], F32, tag="nwt")
                    for _ in range(3):
                        nc.vector.tensor_mul(nwt, rstd, rstd)
                        nc.vector.tensor_mul(nwt, nwt, ve)
                        nc.vector.tensor_scalar(nwt, nwt, -0.5, 1.5, mul,
                                                mybir.AluOpType.add)
                        nc.vector.tensor_mul(rstd, rstd, nwt)
                    rmu = stE.tile([128, 1], F32, tag="rmu")
                    nc.vector.tensor_mul(rmu, mv[:, 0:1], rstd)
                    hn2 = stE.tile([128, C], BF, tag="hn2")
                    nc.vector.tensor_scalar(hn2, o1, rstd, rmu, mul,
                                            mybir.AluOpType.subtract)
                    nc.sync.dma_start_transpose(
                        hn2T[:, :, J * 128:(J + 1) * 128], hn2)
                    nc.vector.tensor_scalar_mul(
                        hn8[:, :, :, J * 128:(J + 1) * 128],
                        hn2T[:, :, J * 128:(J + 1) * 128].rearrange(
                            "p (a b) t -> p a b t", b=2), 16.0)

                emit_attn(0)
                emit_attn(1)
                emit_attn(2)
                emit_attn(3)
                emit_eprep(0)
                emit_eprep(1)
                emit_eprep(2)

            # ---------------- FFN: SwiGLU (bf16, tanh-silu) ----------
            with tc.tile_pool(name="wstream", bufs=6) as wstream, \
                 tc.tile_pool(name="gtmp", bufs=3) as gtmp, \
                 tc.tile_pool(name="psG", bufs=3, space="PSUM") as psG, \
                 tc.tile_pool(name="psW3", bufs=1, space="PSUM") as psW3:
                nc.sync.dma_start(out=w3akeep, in_=w3a[:])
                nc.sync.dma_start(out=w3keep, in_=w3b[:])

                def emit_z(half):
                    hsl = slice(half * 256, half * 256 + 256)
                    for fi in range(NF):
                        wt = wstream.tile([128, NK2, 2, 256], FP8, tag="wt")
                        nc.scalar.dma_start(out=wt, in_=wff[fi])
                        z1 = psG.tile([128, 256], F32, tag="z1")
                        for k2 in range(NK2):
                            nc.tensor.matmul(z1, wt[:, k2, :, 0:128],
                                             hn8[:, k2, :, hsl],
                                             start=(k2 == 0),
                                             stop=(k2 == NK2 - 1),
                                             perf_mode=DRM)
                        th = gtmp.tile([128, 256], BF, tag="th")
                        nc.scalar.activation(out=th, in_=z1, func=AF.Tanh,
                                             scale=thsc[:, 0:1])
                        sil = gtmp.tile([128, 256], BF, tag="sil")
                        nc.vector.scalar_tensor_tensor(
                            out=sil, in0=th, scalar=1.0, in1=z1,
                            op0=mybir.AluOpType.add, op1=mul)
                        z2 = psG.tile([128, 256], F32, tag="z1")
                        for k2 in range(NK2):
                            nc.tensor.matmul(z2, wt[:, k2, :, 128:256],
                                             hn8[:, k2, :, hsl],
                                             start=(k2 == 0),
                                             stop=(k2 == NK2 - 1),
                                             perf_mode=DRM)
                        nc.vector.scalar_tensor_tensor(
                            out=g_sb[:, fi, hsl], in0=sil,
                            scalar=1.0 / 1048576.0, in1=z2, op0=mul, op1=mul)

                def emit_acc(tts):
                    for cw, w3t in ((0, w3akeep), (1, w3keep)):
                        acc = psW3.tile([128, 2, 512], F32, tag="acc",
                                        name="acc")
                        for fi in range(NF):
                            for i, tt in enumerate(tts):
                                nc.tensor.matmul(
                                    acc[:, i, :],
                                    g_sb[:, fi, tt * 128:(tt + 1) * 128],
                                    w3t[:, fi, :],
                                    start=(fi == 0), stop=(fi == NF - 1))
                        for i, tt in enumerate(tts):
                            csl = slice(cw * 512, cw * 512 + 512)
                            nc.vector.tensor_add(xr_sb[:, tt, csl],
                                                 acc[:, i, :], out1[:, tt, csl])
                    for tt in tts:
                        nc.sync.dma_start(
                            out=out[128 * tt:128 * (tt + 1), :].rearrange(
                                "(a p) c -> p a c", a=1),
                            in_=xr_sb[:, tt:tt + 1, :])

                emit_z(0)
                emit_acc([0, 1])
                emit_eprep(3)
                emit_z(1)
                emit_acc([2, 3])
    nc.compile()
    return nc


def _prep(x, Wq, Wk, Wv, Wo, W1, W2, W3, gamma, beta):
    f32 = np.float32
    gcol = gamma.astype(f32)[:, None]
    have_bw = bool(np.any(beta != 0))
    in_maps = []
    for c in range(8):
        b, r = c // 4, c % 4
        hh = [4 * r + h for h in range(HPC)]
        # q/k columns permuted for DR attention layout: col = 32h + (d % 32),
        # jb blocks: (q,slot0),(q,slot1),(k,slot0),(k,slot1)
        wqk_cols = np.empty((C, 4, 128), f32)
        for jbi, (W, slot) in enumerate(
                [(Wq, 0), (Wq, 1), (Wk, 0), (Wk, 1)]):
            for hi, h in enumerate(hh):
                wqk_cols[:, jbi, 32 * hi:32 * hi + 32] = \
                    W[h][:, 32 * slot:32 * slot + 32]
        wqk8 = (gcol[:, None] * wqk_cols * SW).astype(E4)
        # v columns: plain head-major 64h+d
        wv_cols = np.concatenate([Wv[h] for h in hh], axis=1).astype(f32)
        wv8 = (gcol * wv_cols * SW).astype(E4)
        # correction rows from the rounded weights
        gqk8 = (wqk8.astype(f32).sum(axis=0) * (SGWS / SW)).astype(E4)[None]
        gv8 = (wv8.astype(f32).sum(axis=0) * (SGWS / SW)).astype(E4)[None]
        # DR contraction packing [p, k2, ...]: c-dim index = 256*k2+128*slot+p
        wqk_dr = np.ascontiguousarray(
            wqk8.reshape(NK2, 2, 128, 4, 128).transpose(2, 0, 3, 1, 4))
        wv_dr = np.ascontiguousarray(
            wv8.reshape(NK2, 2, 128, 256).transpose(2, 0, 1, 3))
        # Wo rows: local row = 64*hi+d = 128*slot+p
        wo_loc = (Wo[r * 256:(r + 1) * 256, :].astype(f32) * SWO).astype(E4)
        wo_dr = np.ascontiguousarray(
            wo_loc.reshape(2, 128, 2, 512).transpose(1, 2, 0, 3))
        # FFN (bf16, unchanged math)
        w1p = np.zeros((C, HIDP), f32)
        w1p[:, :HID] = W1
        w2p = np.zeros((C, HIDP), f32)
        w2p[:, :HID] = W2
        w3p = np.zeros((HIDP, C), f32)
        w3p[:HID, :] = W3
        w1g = (gcol * w1p).reshape(NKC, 128, NF, 128).transpose(2, 1, 0, 3).reshape(NF, 128, C)
        w2g = (gcol * w2p).reshape(NKC, 128, NF, 128).transpose(2, 1, 0, 3).reshape(NF, 128, C)
        w3r = w3p.reshape(NF, 128, C)
        # [NF, 128p, k2, slot, (w1 128 | 0.5*w2 128)] fp8, x64
        # w1g[fi] dims: [c%128 partition, (kc, f) flattened]
        wff = np.empty((NF, 128, NK2, 2, 256), np.float32)
        for fi in range(NF):
            for proj, wsrc in ((0, w1g), (1, 0.5 * w2g)):
                # [c%128, kc, f] -> group kc into (k2, slot)
                wr = wsrc[fi].reshape(128, NK2, 2, 128) * SW
                wff[fi, :, :, :, proj * 128:(proj + 1) * 128] = wr
        w3ab = np.ascontiguousarray(
            w3r[:, :, 0:512].transpose(1, 0, 2)).astype(BF16)
        w3bb = np.ascontiguousarray(
            w3r[:, :, 512:1024].transpose(1, 0, 2)).astype(BF16)
        xb = x[b].astype(f32)
        # x^T fp8 in DR layout [p, k2, tq, slot, 512]
        xs = np.ascontiguousarray(xb.T * SX).astype(E4)  # [C, T]
        xT_dr = np.ascontiguousarray(
            xs.reshape(NK2, 2, 128, NJ, 512).transpose(2, 0, 3, 1, 4))
        xres = np.concatenate(
            [xb[512 * J + 128 * r: 512 * J + 128 * (r + 1)] for J in range(NJ)])
        m = {
            "xT": xT_dr,
            "xres": np.ascontiguousarray(xres),
            "wqk": wqk_dr,
            "wv": wv_dr,
            "gqk": np.ascontiguousarray(gqk8.reshape(1, 4, 128)),
            "gv": gv8,
            "wo": wo_dr,
            "wff": wff.astype(E4),
            "w3a": w3ab,
            "w3b": w3bb,
        }
        if have_bw:
            bqk = (beta.astype(f32) @ wqk_cols.reshape(C, 512)) * 8.0
            m["bwqk"] = bqk.astype(E4).reshape(1, 4, 128)
            m["bwv"] = ((beta.astype(f32) @ wv_cols) * 8.0).astype(E4)[None]
            m["bw1"] = (beta.astype(f32) @ w1p).astype(f32)
            m["bw2"] = (beta.astype(f32) @ w2p).astype(f32)
        in_maps.append(m)
    return in_maps, have_bw


def kernel(x, Wq, Wk, Wv, Wo, W1, W2, W3, gamma, beta, _bench=None,
           _debug=False):
    x = np.asarray(x)
    in_maps, have_bw = _prep(np.asarray(x), np.asarray(Wq), np.asarray(Wk),
                             np.asarray(Wv), np.asarray(Wo), np.asarray(W1),
                             np.asarray(W2), np.asarray(W3),
                             np.asarray(gamma), np.asarray(beta))
    key = ("k", have_bw, _debug)
    if key not in _cache:
        _cache[key] = _build(have_bw, debug=_debug)
    nc = _cache[key]
    kw = dict(_bench) if _bench else {}
    res = run_bass_kernel_spmd(nc, in_maps, list(range(8)), **kw)
    outf = np.empty((B, T, C), np.float32)
    for c in range(8):
        b, r = c // 4, c % 4
        o = res.results[c]["out"]
        for J in range(NJ):
            outf[b, 512 * J + 128 * r: 512 * J + 128 * (r + 1)] = \
                o[128 * J:128 * (J + 1)]
    if _bench is not None:
        kernel.last_results = res
    global _last_res
    _last_res = res
    return outf
